# revision 47
# baseline (speedup 1.0000x reference)
"""Multi-head attention Trainium2 Bass kernel.

Problem: x[8,1024,768], qkv_w[2304,768], qkv_b[2304], proj_w[768,768],
proj_b[768] -> out[8,1024,768]  (12 heads, head_dim 64, softmax scale 1/8).

Sharding: data-parallel over the batch dim — one batch element per
NeuronCore, 8 cores, no collectives.

Per-core pipeline:
  1. Loads are gpsimd (SWDGE) casting DMAs: f32 DRAM -> bf16 SBUF in
     flight.  XBAR transposes produce xT[c,n], wT[c,d'], pwT[c,c'] bf16.
     Concurrent XBAR transposes on two HWDGE queues corrupt data (shared
     XBAR, no cross-queue guard), so ALL transposes ride the sync queue;
     only x, the v-chunk, and wq/wk tile 0 gate the start of attention —
     the remaining w/proj_w transposes trickle in as jt-loop fillers.
  2. Q/K projections run in fp8e4 DoubleRow perf mode (contraction 256
     per matmul, 2 MACs/cell/cycle); their quantization error washes out
     through the softmax.  V / PV / proj stay bf16 — fp8 there feeds the
     output directly and costs ~1.5% rel error per stage.
  3. Attention per head h (tile t=h//2, partition half (h%2)*64):
     scores sT[j,i] = kT.T@qT bf16 (2-bank PSUM tiles, 2-deep ring), exp
     on ACT with the 1/8 scale folded in (scores absmax ~2.7, no
     max-sub) -> e bf16; pv accumulates outT[d+1,i] += [v|1].T @ e — the
     ones column yields the softmax denominator row for free.  ACT exp
     (~1.15us per [128,1024] tile) and PE trade the critical path; q/k
     work for tile t+1 is interleaved into the jt loops so the PE queue
     never starves the exp stream.
  4. Per-head epilogue, pipelined one head deep: DVE reciprocal of the
     denominator row, DRAM-bounce broadcast to 64 partitions (sync
     queue), normalize multiply emitted one head later (so the DVE queue
     never blocks on the bounce RTT).
  5. proj: out[n,c'] = attnT.T @ pwT bf16, bias on DVE, DMA out
     alternating queues.
"""

import sys

if "/opt/trn_rl_repo" not in sys.path:
    sys.path.insert(0, "/opt/trn_rl_repo")

from contextlib import ExitStack

import numpy as np

import concourse.bass as bass
import concourse.tile as tile
from concourse import mybir
from concourse.bass_utils import run_bass_kernel_spmd
from concourse.masks import make_identity

F32 = mybir.dt.float32
BF16 = mybir.dt.bfloat16
FP8 = mybir.dt.float8e4
AF = mybir.ActivationFunctionType
DR = mybir.MatmulPerfMode.DoubleRow


def _split_dma_waits(nc: bass.Bass):
    """TRN2 instruction encodings hold at most 1 sync-wait (EventSemaphore: 2),
    but Tile can attach several (producer + xbar-mode serialization guards).
    Hoist all but one wait onto single-wait NoOps inserted just before on the
    same engine — same-sequencer FIFO order makes this equivalent.
    """
    for f in nc.m.functions:
        for blk in f.blocks:
            insts = blk.instructions
            i = 0
            while i < len(insts):
                inst = insts[i]
                limit = 2 if isinstance(inst, mybir.InstEventSemaphore) else 1
                if (inst.sync_info is not None
                        and len(inst.sync_info.on_wait) > limit):
                    waits = list(inst.sync_info.on_wait)
                    inst.sync_info = mybir.SyncInfo(
                        on_wait=waits[-limit:],
                        on_update=list(inst.sync_info.on_update))
                    for w in waits[:-limit]:
                        nop = mybir.InstNoOp(
                            name=nc.get_next_instruction_name(),
                            ins=[], outs=[])
                        nop.engine = inst.engine
                        nop.sync_info = mybir.SyncInfo(
                            on_wait=[w], on_update=[])
                        insts.insert(i, nop)
                        i += 1
                i += 1

DEBUG_TAPS = False

B, N, C = 8, 1024, 768
H, HD = 12, 64
D3 = 3 * C  # 2304
SCALE = HD ** -0.5
NT = N // 128   # 8  token tiles
CT = C // 128   # 6  channel tiles


def build_kernel(nc: bass.Bass):
    x = nc.dram_tensor("x", [N, C], F32, kind="ExternalInput").ap()
    qkv_w = nc.dram_tensor("qkv_w", [D3, C], F32, kind="ExternalInput").ap()
    qkv_b = nc.dram_tensor("qkv_b", [D3], F32, kind="ExternalInput").ap()
    proj_w = nc.dram_tensor("proj_w", [C, C], F32, kind="ExternalInput").ap()
    proj_b = nc.dram_tensor("proj_b", [C], F32, kind="ExternalInput").ap()
    out = nc.dram_tensor("out", [N, C], F32, kind="ExternalOutput").ap()

    def bcast_ap(src: bass.AP, parts: int) -> bass.AP:
        # partition-broadcast a 1-D DRAM row: ap [[0, parts], [1, n]]
        return bass.AP(tensor=src.tensor, offset=src.offset,
                       ap=[[0, parts], *src.ap])

    with tile.TileContext(nc) as tc, ExitStack() as ctx:
        consts = ctx.enter_context(tc.tile_pool(name="consts", bufs=1))
        stage = ctx.enter_context(tc.tile_pool(name="stage", bufs=2))
        expp = ctx.enter_context(tc.tile_pool(name="expp", bufs=3))
        outp = ctx.enter_context(tc.tile_pool(name="outp", bufs=2))
        ps_q = ctx.enter_context(tc.tile_pool(name="ps_q", bufs=2, space="PSUM"))
        ps_s = ctx.enter_context(tc.tile_pool(name="ps_s", bufs=2, space="PSUM"))
        ps_o = ctx.enter_context(tc.tile_pool(name="ps_o", bufs=1, space="PSUM"))
        dram = ctx.enter_context(tc.tile_pool(name="dram", bufs=1, space="DRAM"))

        # ---- persistent operands -------------------------------------
        xT8 = consts.tile([128, CT, N], FP8)        # x.T   [c, n] (q/k DR)
        wT8 = consts.tile([128, CT, 2 * C], FP8)    # qkv_w.T q,k rows (DR)
        qTt = consts.tile([128, CT, N], BF16)       # q.T  [d, n] (+bias)
        kTt = consts.tile([128, CT, N], BF16)       # k.T  [d, n] (+bias)
        v_sb = consts.tile([128, NT, H, HD + 1], BF16)  # v natural + ones
        attnU = consts.tile([128, CT, N], BF16)     # attn.T (normalized
        attnT = attnU                               # in place per head)
        qkb = consts.tile([128, 2 * CT], F32)       # q,k bias per-partition
        vb_bc = consts.tile([128, C], F32)          # v bias bcast
        pjb_bc = consts.tile([128, C], F32)         # proj bias bcast
        dscratch = dram.tile([H, N], F32)           # DRAM bounce: denom
        dscratch2 = dram.tile([H, N], F32)          # DRAM bounce: 1/denom

        # ---- biases ---------------------------------------------------
        nc.sync.dma_start(out=qkb, in_=qkv_b[0:2 * C].rearrange(
            "(t p) -> p t", p=128))
        nc.sync.dma_start(out=vb_bc, in_=bcast_ap(qkv_b[2 * C:D3], 128))
        nc.sync.dma_start(out=pjb_bc, in_=bcast_ap(proj_b, 128))
        nc.vector.memset(v_sb[:, :, :, HD:HD + 1], 1.0)

        # ---- loads (gpsimd casting DMA f32->bf16 in flight) ----------
        # x is loaded with the PERMUTED token order n = p*8 + t (contiguous
        # 24KB runs per partition -> fast SWDGE).  The permutation is
        # consistent through the whole attention pipeline (softmax sums
        # over all j regardless of order) and is inverted by the final
        # output-DMA scatter.  w/proj_w keep d' structure (heads!) and
        # load per-tile so wq/wk tile 0 arrives early.
        dummy = stage.tile([1, 8], F32, tag="dummy", bufs=1)
        nc.scalar.activation(out=dummy, in_=qkb[0:1, 0:8], func=AF.Exp)
        x_sb = stage.tile([128, NT, C], BF16, tag="x_sb", bufs=1)
        nc.gpsimd.dma_start(out=x_sb, in_=x.rearrange("(p t) c -> p t c", t=NT))
        w_sbs = {}
        for roff in (2 * C, 0, C):
            w_sbs[roff] = stage.tile([128, CT, C], BF16, tag=f"w_sb{roff}",
                                     bufs=1, name=f"w{roff}")

        def w_load(roff, j):
            nc.gpsimd.dma_start(
                out=w_sbs[roff][:, j, :],
                in_=qkv_w[roff + j * 128:roff + (j + 1) * 128, :])

        ident = consts.tile([128, 128], F32)
        make_identity(nc, ident)
        w32 = {}
        for nm, roff in (("q0", 0), ("k0", C), ("v0", 2 * C),
                         ("v1", 2 * C + 128)):
            w32[nm] = stage.tile([128, C], F32, tag=f"w32{nm}", bufs=1,
                                 name=f"w32{nm}")
            nc.gpsimd.dma_start(out=w32[nm],
                                in_=qkv_w[roff:roff + 128, :])
        for j in range(2, CT):
            w_load(2 * C, j)           # v rows 2-5
        for roff in (0, C):            # q/k tiles 1-5, one DMA each
            nc.gpsimd.dma_start(
                out=w_sbs[roff][:, 1:CT, :],
                in_=qkv_w[roff + 128:roff + C, :].rearrange(
                    "(t p) c -> p t c", p=128))
        pw_sb = stage.tile([128, CT, C], BF16, tag="pw_sb", bufs=1,
                           name="pw_sb")
        nc.gpsimd.dma_start(
            out=pw_sb, in_=proj_w.rearrange("(t p) c -> p t c", p=128))

        # ---- transposes (single sync queue — see module docstring) ---
        def xpose(dst_ap, src_ap):
            nc.sync.dma_start_transpose(out=dst_ap, in_=src_ap)

        # q/k chunks: [c-part, ct, d'-local] (canonical, per-tile writes)
        wT_bfs = {}
        for roff in (0, C):
            wT_bfs[roff] = stage.tile([128, CT, C], BF16, tag=f"wT_bf{roff}",
                                      bufs=1, name=f"wT{roff}")
        # v / proj_w transposed wholesale: 4D [c-part, j, ct, q] fold
        # (d' = j*128 + q); matmul rhs slices are [:, lo//128:hi//128, ct, :]
        wTv_bf = stage.tile([128, CT, CT, 128], BF16, tag="wTv_bf", bufs=1)
        pwT_bf = stage.tile([128, CT, CT, 128], BF16, tag="pwT_bf", bufs=1,
                            name="pwT_bf")

        def w_xpose(roff, j):
            # transpose w rows [roff+128j, roff+128(j+1)) and cast the
            # fresh slice to fp8 for the DoubleRow q/k matmuls
            xpose(wT_bfs[roff][:, :, j * 128:(j + 1) * 128],
                  w_sbs[roff][:, j, :])
            # cast on the idle gpsimd engine: a DVE-queued cast would
            # head-of-line-block the inline V bias adds behind XBAR waits
            nc.gpsimd.tensor_copy(
                out=wT8[:, :, roff + j * 128:roff + (j + 1) * 128],
                in_=wT_bfs[roff][:, :, j * 128:(j + 1) * 128])

        # x: transposed in halves so q0/k0 ic0 can start after half A
        # (n' = t*128 + q relabels the permuted tokens)
        xT_bf = stage.tile([128, NT, CT, 128], BF16, tag="xT_bf", bufs=1)
        xT8v = xT8.rearrange("p c (t q) -> p t c q", q=128)
        # q0/k0/v0/v1 via PE identity-transpose (f32, idle PE + spare
        # F32 PSUM ring) so the serialized XBAR prefix is x alone
        for nm, dst in (("q0", wT8[:, :, 0:128]), ("k0", wT8[:, :, C:C + 128]),
                        ("v0", wTv_bf[:, 0]), ("v1", wTv_bf[:, 1])):
            for c0, w in ((0, 4), (4, 2)):
                pst = ps_q.tile([128, 512], F32, tag="ps1b", name="pstw")
                for k in range(w):
                    nc.tensor.transpose(
                        out=pst[:, k * 128:(k + 1) * 128],
                        in_=w32[nm][:, (c0 + k) * 128:(c0 + k + 1) * 128],
                        identity=ident)
                nc.vector.tensor_copy(
                    out=dst[:, c0:c0 + w, :] if nm in ("v0", "v1")
                    else dst[:, c0:c0 + w, 0:128],
                    in_=pst[:, 0:w * 128].rearrange("p (c q) -> p c q", q=128))
        xpose(xT_bf[:, 0:4].rearrange("p t c q -> p (t c) q"),
              x_sb[:, 0:4, :].rearrange("p t c -> p (t c)"))
        nc.vector.tensor_copy(out=xT8v[:, 0:4], in_=xT_bf[:, 0:4])
        xpose(xT_bf[:, 4:8].rearrange("p t c q -> p (t c) q"),
              x_sb[:, 4:8, :].rearrange("p t c -> p (t c)"))
        nc.vector.tensor_copy(out=xT8v[:, 4:8], in_=xT_bf[:, 4:8])
        xpose(wTv_bf[:, 2, :, :], w_sbs[2 * C][:, 2, :])
        xpose(wTv_bf[:, 3, :, :], w_sbs[2 * C][:, 3, :])
        for j in range(4, CT):         # v rows 4-5 via XBAR
            xpose(wTv_bf[:, j, :, :], w_sbs[2 * C][:, j, :])

        # ---- Q/K projection units (fp8 DoubleRow, transposed out) ----
        def qk_unit(t, is_k, ic):
            # qkvT[d', n] = wT.T @ xT for d' tile t (+C if k), n chunk ic
            woff = (C if is_k else 0) + t * 128
            dst = kTt if is_k else qTt
            ps = ps_q.tile([128, 512], F32, tag="ps1b", name="psqk")
            for k in range(3):
                nc.tensor.matmul(
                    ps,
                    lhsT=wT8[:, 2 * k:2 * k + 2, woff:woff + 128],
                    rhs=xT8[:, 2 * k:2 * k + 2, ic * 512:(ic + 1) * 512],
                    start=(k == 0), stop=(k == 2), perf_mode=DR)
            nc.vector.tensor_scalar_add(
                out=dst[:, t, ic * 512:(ic + 1) * 512], in0=ps,
                scalar1=qkb[:, CT * is_k + t:CT * is_k + t + 1])

        for ic in range(2):
            qk_unit(0, False, ic)
        for ic in range(2):
            qk_unit(0, True, ic)

        # ---- V projection units (bf16), natural [n, (h, d)] ----------
        # chunk A (heads 0-7) is emitted inline in head 0's jt loop just
        # before each pv that consumes it; chunk B rides pair-1/2 fillers
        def v_unit(t, chunk):
            lo, hi = (0, 512) if chunk == 0 else (512, 768)
            psv = ps_q.tile([128, 512], F32, tag="ps1b", name="psv")
            for ct in range(CT):
                nc.tensor.matmul(
                    psv[:, 0:hi - lo],
                    lhsT=xT_bf[:, t, ct, :],
                    rhs=wTv_bf[:, lo // 128:hi // 128, ct, :],
                    start=(ct == 0), stop=(ct == CT - 1))
            nc.vector.tensor_add(
                out=v_sb[:, t, lo // HD:hi // HD, 0:HD],
                in0=psv[:, 0:hi - lo].rearrange("p (h d) -> p h d", d=HD),
                in1=vb_bc[:, lo:hi].rearrange("p (h d) -> p h d", d=HD))

        for tau in range(NT):
            v_unit(tau, 0)

        # ---- attention ------------------------------------------------
        fillers = []
        prev_mul = None  # deferred normalize multiply (h-1)

        for t in range(CT):
            # enqueue transposes + q/k units for tile t+1 into this
            # tile's 8 odd-jt filler slots
            if t < CT - 1:
                if t in (1, 2):
                    for tau in range(4 * (t - 1), 4 * t):
                        fillers.append(lambda tt=tau: v_unit(tt, 1))
                fillers.append(lambda tt=t + 1: w_xpose(0, tt))
                fillers.append(lambda tt=t + 1: w_xpose(C, tt))
                for is_k in (False, True):
                    for ic in range(2):
                        fillers.append(
                            lambda tt=t + 1, kk=is_k, cc=ic: qk_unit(tt, kk, cc))
                if t < 3:
                    fillers.append(lambda jj=t: xpose(
                        pwT_bf[:, jj, :, :], pw_sb[:, jj, :]))
            else:
                for j in range(3, CT):
                    fillers.append(lambda jj=j: xpose(
                        pwT_bf[:, jj, :, :], pw_sb[:, jj, :]))

            for hh in range(2):
                h = 2 * t + hh
                hb = hh * 64
                o_ps = ps_o.tile([HD + 1, N], F32, tag="o", name="o_ps")
                for jt in range(NT):
                    e_bf = expp.tile([128, N], BF16, tag="e", name="e_bf")
                    s = ps_s.tile([128, N], F32, tag="s", name="s_ps")
                    for ic in range(2):
                        nc.tensor.matmul(
                            s[:, ic * 512:(ic + 1) * 512],
                            lhsT=kTt[hb:hb + 64, t, jt * 128:(jt + 1) * 128],
                            rhs=qTt[hb:hb + 64, t, ic * 512:(ic + 1) * 512],
                            start=True, stop=True)
                    nc.scalar.activation(out=e_bf, in_=s,
                                         func=AF.Exp, scale=SCALE)
                    for ic in range(2):
                        nc.tensor.matmul(
                            o_ps[:, ic * 512:(ic + 1) * 512],
                            lhsT=v_sb[:, jt, h, :],
                            rhs=e_bf[:, ic * 512:(ic + 1) * 512],
                            start=(jt == 0), stop=(jt == NT - 1))
                    if fillers:
                        fillers.pop(0)()

                # epilogue: attnU copy first (frees the PSUM accumulator
                # for the next head), then the denominator round-trip:
                # a [1,N] DVE reciprocal is single-lane (~6.5us!), so
                # bounce the row through DRAM reshaped to [128, N/128]
                # and run the reciprocal on all lanes (~0.2us).
                nc.vector.tensor_copy(
                    out=attnU[hb:hb + 64, t, :], in_=o_ps[0:HD, :])
                den = stage.tile([1, N], F32, tag="den")
                nc.vector.tensor_copy(out=den, in_=o_ps[HD:HD + 1, :])
                nc.sync.dma_start(out=dscratch[h:h + 1, :], in_=den)
                den128 = stage.tile([128, N // 128], F32, tag="den128")
                nc.sync.dma_start(out=den128, in_=dscratch[h, :].rearrange(
                    "(p a) -> p a", p=128))
                den128r = stage.tile([128, N // 128], F32, tag="den128r")
                nc.vector.reciprocal(out=den128r, in_=den128)
                nc.sync.dma_start(out=dscratch2[h, :].rearrange(
                    "(p a) -> p a", p=128), in_=den128r)
                rbc = stage.tile([128, N], F32, tag="rbc")
                nc.sync.dma_start(out=rbc[hb:hb + 64, :],
                                  in_=bcast_ap(dscratch2[h, :], 64))
                if prev_mul is not None:
                    prev_mul()
                prev_mul = (lambda hb=hb, t=t, rbc=rbc: nc.vector.tensor_mul(
                    out=attnT[hb:hb + 64, t, :],
                    in0=attnU[hb:hb + 64, t, :], in1=rbc[hb:hb + 64, :]))
        prev_mul()

        if DEBUG_TAPS:
            taps = {
                "dbg_q": (qTt, BF16), "dbg_k": (kTt, BF16),
                "dbg_v": (v_sb, BF16), "dbg_attnU": (attnU, BF16),
                "dbg_attnT8": (attnT, BF16), "dbg_x8": (xT8, FP8),
                "dbg_w8": (wT8, FP8), "dbg_pw8": (pwT_bf, BF16),
            }
            for nm, (tl, dt) in taps.items():
                flat = int(np.prod(tl.shape[1:]))
                d = nc.dram_tensor(nm, [128, flat], dt,
                                   kind="ExternalOutput").ap()
                nc.sync.dma_start(out=d, in_=tl)
            dden = nc.dram_tensor("dbg_den", [H, N], F32,
                                  kind="ExternalOutput").ap()
            nc.sync.dma_start(out=dden, in_=dscratch)

        # ---- output projection (bf16) --------------------------------
        for t in range(NT):
            osb = outp.tile([128, C], F32, tag="osb")
            for lo, hi in ((0, 512), (512, 768)):
                psp = ps_q.tile([128, 512], F32, tag="ps1b", name="psp")
                for ct in range(CT):
                    nc.tensor.matmul(
                        psp[:, 0:hi - lo],
                        lhsT=attnT[:, ct, t * 128:(t + 1) * 128],
                        rhs=pwT_bf[:, lo // 128:hi // 128, ct, :],
                        start=(ct == 0), stop=(ct == CT - 1))
                nc.vector.tensor_add(
                    out=osb[:, lo:hi], in0=psp[:, 0:hi - lo],
                    in1=pjb_bc[:, lo:hi])
            eng = nc.sync if t % 2 == 0 else nc.scalar
            eng.dma_start(out=out.rearrange("(p t) c -> p t c", t=NT)[:, t, :],
                          in_=osb)

    _split_dma_waits(nc)
    return nc


_NC_CACHE = None


def _get_nc():
    global _NC_CACHE
    if _NC_CACHE is None:
        _NC_CACHE = build_kernel(
            bass.Bass("TRN2", target_bir_lowering=False, debug=False))
    return _NC_CACHE


def kernel(**inputs: np.ndarray) -> np.ndarray:
    nc = _get_nc()
    x = np.ascontiguousarray(inputs["x"], dtype=np.float32)
    shared = {
        "qkv_w": np.ascontiguousarray(inputs["qkv_w"], dtype=np.float32),
        "qkv_b": np.ascontiguousarray(inputs["qkv_b"], dtype=np.float32),
        "proj_w": np.ascontiguousarray(inputs["proj_w"], dtype=np.float32),
        "proj_b": np.ascontiguousarray(inputs["proj_b"], dtype=np.float32),
    }
    in_maps = [{"x": x[b], **shared} for b in range(B)]
    res = run_bass_kernel_spmd(nc, in_maps, core_ids=list(range(B)))
    return np.stack([r["out"] for r in res.results]).astype(np.float32)


if __name__ == "__main__":
    from reference import setup_inputs, reference

    inputs = {k: np.asarray(v) for k, v in setup_inputs().items()}
    got = kernel(**inputs)
    exp = np.asarray(reference(**inputs))
    err = np.abs(got - exp)
    print("abs err max:", err.max(), "ref absmax:", np.abs(exp).max())
    print("rel(absmax):", err.max() / np.abs(exp).max())


# revision 50
# speedup vs baseline: 1.0010x; 1.0010x over previous
"""Multi-head attention Trainium2 Bass kernel.

Problem: x[8,1024,768], qkv_w[2304,768], qkv_b[2304], proj_w[768,768],
proj_b[768] -> out[8,1024,768]  (12 heads, head_dim 64, softmax scale 1/8).

Sharding: data-parallel over the batch dim — one batch element per
NeuronCore, 8 cores, no collectives.

Per-core pipeline:
  1. Loads are gpsimd (SWDGE) casting DMAs: f32 DRAM -> bf16 SBUF in
     flight.  XBAR transposes produce xT[c,n], wT[c,d'], pwT[c,c'] bf16.
     Concurrent XBAR transposes on two HWDGE queues corrupt data (shared
     XBAR, no cross-queue guard), so ALL transposes ride the sync queue;
     only x, the v-chunk, and wq/wk tile 0 gate the start of attention —
     the remaining w/proj_w transposes trickle in as jt-loop fillers.
  2. Q/K projections run in fp8e4 DoubleRow perf mode (contraction 256
     per matmul, 2 MACs/cell/cycle); their quantization error washes out
     through the softmax.  V / PV / proj stay bf16 — fp8 there feeds the
     output directly and costs ~1.5% rel error per stage.
  3. Attention per head h (tile t=h//2, partition half (h%2)*64):
     scores sT[j,i] = kT.T@qT bf16 (2-bank PSUM tiles, 2-deep ring), exp
     on ACT with the 1/8 scale folded in (scores absmax ~2.7, no
     max-sub) -> e bf16; pv accumulates outT[d+1,i] += [v|1].T @ e — the
     ones column yields the softmax denominator row for free.  ACT exp
     (~1.15us per [128,1024] tile) and PE trade the critical path; q/k
     work for tile t+1 is interleaved into the jt loops so the PE queue
     never starves the exp stream.
  4. Per-head epilogue, pipelined one head deep: DVE reciprocal of the
     denominator row, DRAM-bounce broadcast to 64 partitions (sync
     queue), normalize multiply emitted one head later (so the DVE queue
     never blocks on the bounce RTT).
  5. proj: out[n,c'] = attnT.T @ pwT bf16, bias on DVE, DMA out
     alternating queues.
"""

import sys

if "/opt/trn_rl_repo" not in sys.path:
    sys.path.insert(0, "/opt/trn_rl_repo")

from contextlib import ExitStack

import numpy as np

import concourse.bass as bass
import concourse.tile as tile
from concourse import mybir
from concourse.bass_utils import run_bass_kernel_spmd
from concourse.masks import make_identity

F32 = mybir.dt.float32
BF16 = mybir.dt.bfloat16
FP8 = mybir.dt.float8e4
AF = mybir.ActivationFunctionType
DR = mybir.MatmulPerfMode.DoubleRow


def _split_dma_waits(nc: bass.Bass):
    """TRN2 instruction encodings hold at most 1 sync-wait (EventSemaphore: 2),
    but Tile can attach several (producer + xbar-mode serialization guards).
    Hoist all but one wait onto single-wait NoOps inserted just before on the
    same engine — same-sequencer FIFO order makes this equivalent.
    """
    for f in nc.m.functions:
        for blk in f.blocks:
            insts = blk.instructions
            i = 0
            while i < len(insts):
                inst = insts[i]
                limit = 2 if isinstance(inst, mybir.InstEventSemaphore) else 1
                if (inst.sync_info is not None
                        and len(inst.sync_info.on_wait) > limit):
                    waits = list(inst.sync_info.on_wait)
                    inst.sync_info = mybir.SyncInfo(
                        on_wait=waits[-limit:],
                        on_update=list(inst.sync_info.on_update))
                    for w in waits[:-limit]:
                        nop = mybir.InstNoOp(
                            name=nc.get_next_instruction_name(),
                            ins=[], outs=[])
                        nop.engine = inst.engine
                        nop.sync_info = mybir.SyncInfo(
                            on_wait=[w], on_update=[])
                        insts.insert(i, nop)
                        i += 1
                i += 1

DEBUG_TAPS = False

B, N, C = 8, 1024, 768
H, HD = 12, 64
D3 = 3 * C  # 2304
SCALE = HD ** -0.5
NT = N // 128   # 8  token tiles
CT = C // 128   # 6  channel tiles


def build_kernel(nc: bass.Bass):
    x = nc.dram_tensor("x", [N, C], F32, kind="ExternalInput").ap()
    qkv_w = nc.dram_tensor("qkv_w", [D3, C], F32, kind="ExternalInput").ap()
    qkv_b = nc.dram_tensor("qkv_b", [D3], F32, kind="ExternalInput").ap()
    proj_w = nc.dram_tensor("proj_w", [C, C], F32, kind="ExternalInput").ap()
    proj_b = nc.dram_tensor("proj_b", [C], F32, kind="ExternalInput").ap()
    out = nc.dram_tensor("out", [N, C], F32, kind="ExternalOutput").ap()

    def bcast_ap(src: bass.AP, parts: int) -> bass.AP:
        # partition-broadcast a 1-D DRAM row: ap [[0, parts], [1, n]]
        return bass.AP(tensor=src.tensor, offset=src.offset,
                       ap=[[0, parts], *src.ap])

    with tile.TileContext(nc) as tc, ExitStack() as ctx:
        consts = ctx.enter_context(tc.tile_pool(name="consts", bufs=1))
        stage = ctx.enter_context(tc.tile_pool(name="stage", bufs=2))
        expp = ctx.enter_context(tc.tile_pool(name="expp", bufs=4))
        outp = ctx.enter_context(tc.tile_pool(name="outp", bufs=2))
        ps_q = ctx.enter_context(tc.tile_pool(name="ps_q", bufs=2, space="PSUM"))
        ps_s = ctx.enter_context(tc.tile_pool(name="ps_s", bufs=2, space="PSUM"))
        ps_o = ctx.enter_context(tc.tile_pool(name="ps_o", bufs=1, space="PSUM"))
        dram = ctx.enter_context(tc.tile_pool(name="dram", bufs=1, space="DRAM"))

        # ---- persistent operands -------------------------------------
        xT8 = consts.tile([128, CT, N], FP8)        # x.T   [c, n] (q/k DR)
        wT8 = consts.tile([128, CT, 2 * C], FP8)    # qkv_w.T q,k rows (DR)
        qTt = consts.tile([128, CT, N], BF16)       # q.T  [d, n] (+bias)
        kTt = consts.tile([128, CT, N], BF16)       # k.T  [d, n] (+bias)
        v_sb = consts.tile([128, NT, H, HD + 1], BF16)  # v natural + ones
        attnU = consts.tile([128, CT, N], BF16)     # attn.T (normalized
        attnT = attnU                               # in place per head)
        qkb = consts.tile([128, 2 * CT], F32)       # q,k bias per-partition
        vb_bc = consts.tile([128, C], F32)          # v bias bcast
        pjb_bc = consts.tile([128, C], F32)         # proj bias bcast
        dscratch = dram.tile([H, N], F32)           # DRAM bounce: denom
        dscratch2 = dram.tile([H, N], F32)          # DRAM bounce: 1/denom

        # ---- biases ---------------------------------------------------
        nc.sync.dma_start(out=qkb, in_=qkv_b[0:2 * C].rearrange(
            "(t p) -> p t", p=128))
        nc.sync.dma_start(out=vb_bc, in_=bcast_ap(qkv_b[2 * C:D3], 128))
        nc.sync.dma_start(out=pjb_bc, in_=bcast_ap(proj_b, 128))
        nc.vector.memset(v_sb[:, :, :, HD:HD + 1], 1.0)

        # ---- loads (gpsimd casting DMA f32->bf16 in flight) ----------
        # x is loaded with the PERMUTED token order n = p*8 + t (contiguous
        # 24KB runs per partition -> fast SWDGE).  The permutation is
        # consistent through the whole attention pipeline (softmax sums
        # over all j regardless of order) and is inverted by the final
        # output-DMA scatter.  w/proj_w keep d' structure (heads!) and
        # load per-tile so wq/wk tile 0 arrives early.
        dummy = stage.tile([1, 1], F32, tag="dummy", bufs=1)
        nc.scalar.activation(out=dummy, in_=qkb[0:1, 0:1], func=AF.Exp)
        x_sb = stage.tile([128, NT, C], BF16, tag="x_sb", bufs=1)
        nc.gpsimd.dma_start(out=x_sb, in_=x.rearrange("(p t) c -> p t c", t=NT))
        w_sbs = {}
        for roff in (2 * C, 0, C):
            w_sbs[roff] = stage.tile([128, CT, C], BF16, tag=f"w_sb{roff}",
                                     bufs=1, name=f"w{roff}")

        def w_load(roff, j):
            nc.gpsimd.dma_start(
                out=w_sbs[roff][:, j, :],
                in_=qkv_w[roff + j * 128:roff + (j + 1) * 128, :])

        ident = consts.tile([128, 128], F32)
        make_identity(nc, ident)
        w32 = {}
        for nm, roff in (("q0", 0), ("k0", C), ("v0", 2 * C),
                         ("v1", 2 * C + 128)):
            w32[nm] = stage.tile([128, C], F32, tag=f"w32{nm}", bufs=1,
                                 name=f"w32{nm}")
            nc.gpsimd.dma_start(out=w32[nm],
                                in_=qkv_w[roff:roff + 128, :])
        for j in range(2, CT):
            w_load(2 * C, j)           # v rows 2-5
        for roff in (0, C):            # q/k tiles 1-5, one DMA each
            nc.gpsimd.dma_start(
                out=w_sbs[roff][:, 1:CT, :],
                in_=qkv_w[roff + 128:roff + C, :].rearrange(
                    "(t p) c -> p t c", p=128))
        pw_sb = stage.tile([128, CT, C], BF16, tag="pw_sb", bufs=1,
                           name="pw_sb")
        nc.gpsimd.dma_start(
            out=pw_sb, in_=proj_w.rearrange("(t p) c -> p t c", p=128))

        # ---- transposes (single sync queue — see module docstring) ---
        def xpose(dst_ap, src_ap):
            nc.sync.dma_start_transpose(out=dst_ap, in_=src_ap)

        # q/k chunks: [c-part, ct, d'-local] (canonical, per-tile writes)
        wT_bfs = {}
        for roff in (0, C):
            wT_bfs[roff] = stage.tile([128, CT, C], BF16, tag=f"wT_bf{roff}",
                                      bufs=1, name=f"wT{roff}")
        # v / proj_w transposed wholesale: 4D [c-part, j, ct, q] fold
        # (d' = j*128 + q); matmul rhs slices are [:, lo//128:hi//128, ct, :]
        wTv_bf = stage.tile([128, CT, CT, 128], BF16, tag="wTv_bf", bufs=1)
        pwT_bf = stage.tile([128, CT, CT, 128], BF16, tag="pwT_bf", bufs=1,
                            name="pwT_bf")

        def w_xpose(roff, j):
            # transpose w rows [roff+128j, roff+128(j+1)) and cast the
            # fresh slice to fp8 for the DoubleRow q/k matmuls
            xpose(wT_bfs[roff][:, :, j * 128:(j + 1) * 128],
                  w_sbs[roff][:, j, :])
            # cast on the idle gpsimd engine: a DVE-queued cast would
            # head-of-line-block the inline V bias adds behind XBAR waits
            nc.gpsimd.tensor_copy(
                out=wT8[:, :, roff + j * 128:roff + (j + 1) * 128],
                in_=wT_bfs[roff][:, :, j * 128:(j + 1) * 128])

        # x: transposed in halves so q0/k0 ic0 can start after half A
        # (n' = t*128 + q relabels the permuted tokens)
        xT_bf = stage.tile([128, NT, CT, 128], BF16, tag="xT_bf", bufs=1)
        xT8v = xT8.rearrange("p c (t q) -> p t c q", q=128)
        # q0/k0/v0/v1 via PE identity-transpose (f32, idle PE + spare
        # F32 PSUM ring) so the serialized XBAR prefix is x alone
        for nm, dst in (("q0", wT8[:, :, 0:128]), ("k0", wT8[:, :, C:C + 128]),
                        ("v0", wTv_bf[:, 0]), ("v1", wTv_bf[:, 1])):
            for c0, w in ((0, 4), (4, 2)):
                pst = ps_q.tile([128, 512], F32, tag="ps1b", name="pstw")
                for k in range(w):
                    nc.tensor.transpose(
                        out=pst[:, k * 128:(k + 1) * 128],
                        in_=w32[nm][:, (c0 + k) * 128:(c0 + k + 1) * 128],
                        identity=ident)
                nc.vector.tensor_copy(
                    out=dst[:, c0:c0 + w, :] if nm in ("v0", "v1")
                    else dst[:, c0:c0 + w, 0:128],
                    in_=pst[:, 0:w * 128].rearrange("p (c q) -> p c q", q=128))
        xpose(xT_bf[:, 0:4].rearrange("p t c q -> p (t c) q"),
              x_sb[:, 0:4, :].rearrange("p t c -> p (t c)"))
        nc.vector.tensor_copy(out=xT8v[:, 0:4], in_=xT_bf[:, 0:4])
        xpose(wTv_bf[:, 2, :, :], w_sbs[2 * C][:, 2, :])
        xpose(wTv_bf[:, 3, :, :], w_sbs[2 * C][:, 3, :])
        xpose(xT_bf[:, 4:8].rearrange("p t c q -> p (t c) q"),
              x_sb[:, 4:8, :].rearrange("p t c -> p (t c)"))
        nc.vector.tensor_copy(out=xT8v[:, 4:8], in_=xT_bf[:, 4:8])
        for j in range(4, CT):         # v rows 4-5 via XBAR
            xpose(wTv_bf[:, j, :, :], w_sbs[2 * C][:, j, :])

        # ---- Q/K projection units (fp8 DoubleRow, transposed out) ----
        def qk_unit(t, is_k, ic):
            # qkvT[d', n] = wT.T @ xT for d' tile t (+C if k), n chunk ic
            woff = (C if is_k else 0) + t * 128
            dst = kTt if is_k else qTt
            ps = ps_q.tile([128, 512], F32, tag="ps1b", name="psqk")
            for k in range(3):
                nc.tensor.matmul(
                    ps,
                    lhsT=wT8[:, 2 * k:2 * k + 2, woff:woff + 128],
                    rhs=xT8[:, 2 * k:2 * k + 2, ic * 512:(ic + 1) * 512],
                    start=(k == 0), stop=(k == 2), perf_mode=DR)
            nc.vector.tensor_scalar_add(
                out=dst[:, t, ic * 512:(ic + 1) * 512], in0=ps,
                scalar1=qkb[:, CT * is_k + t:CT * is_k + t + 1])

        for ic in range(2):
            qk_unit(0, False, ic)
        for ic in range(2):
            qk_unit(0, True, ic)

        # ---- V projection units (bf16), natural [n, (h, d)] ----------
        # chunk A (heads 0-7) is emitted inline in head 0's jt loop just
        # before each pv that consumes it; chunk B rides pair-1/2 fillers
        def v_unit(t, chunk):
            lo, hi = (0, 512) if chunk == 0 else (512, 768)
            psv = ps_q.tile([128, 512], F32, tag="ps1b", name="psv")
            for ct in range(CT):
                nc.tensor.matmul(
                    psv[:, 0:hi - lo],
                    lhsT=xT_bf[:, t, ct, :],
                    rhs=wTv_bf[:, lo // 128:hi // 128, ct, :],
                    start=(ct == 0), stop=(ct == CT - 1))
            nc.vector.tensor_add(
                out=v_sb[:, t, lo // HD:hi // HD, 0:HD],
                in0=psv[:, 0:hi - lo].rearrange("p (h d) -> p h d", d=HD),
                in1=vb_bc[:, lo:hi].rearrange("p (h d) -> p h d", d=HD))

        for tau in range(NT):
            v_unit(tau, 0)

        # ---- attention ------------------------------------------------
        fillers = []
        prev_mul = None  # deferred normalize multiply (h-1)

        for t in range(CT):
            # enqueue transposes + q/k units for tile t+1 into this
            # tile's 8 odd-jt filler slots
            if t < CT - 1:
                if t in (1, 2):
                    for tau in range(4 * (t - 1), 4 * t):
                        fillers.append(lambda tt=tau: v_unit(tt, 1))
                fillers.append(lambda tt=t + 1: w_xpose(0, tt))
                fillers.append(lambda tt=t + 1: w_xpose(C, tt))
                for is_k in (False, True):
                    for ic in range(2):
                        fillers.append(
                            lambda tt=t + 1, kk=is_k, cc=ic: qk_unit(tt, kk, cc))
                if t < 3:
                    fillers.append(lambda jj=t: xpose(
                        pwT_bf[:, jj, :, :], pw_sb[:, jj, :]))
            else:
                for j in range(3, CT):
                    fillers.append(lambda jj=j: xpose(
                        pwT_bf[:, jj, :, :], pw_sb[:, jj, :]))

            for hh in range(2):
                h = 2 * t + hh
                hb = hh * 64
                o_ps = ps_o.tile([HD + 1, N], F32, tag="o", name="o_ps")
                for jt in range(NT):
                    e_bf = expp.tile([128, N], BF16, tag="e", name="e_bf")
                    s = ps_s.tile([128, N], F32, tag="s", name="s_ps")
                    for ic in range(2):
                        nc.tensor.matmul(
                            s[:, ic * 512:(ic + 1) * 512],
                            lhsT=kTt[hb:hb + 64, t, jt * 128:(jt + 1) * 128],
                            rhs=qTt[hb:hb + 64, t, ic * 512:(ic + 1) * 512],
                            start=True, stop=True)
                    nc.scalar.activation(out=e_bf, in_=s,
                                         func=AF.Exp, scale=SCALE)
                    for ic in range(2):
                        nc.tensor.matmul(
                            o_ps[:, ic * 512:(ic + 1) * 512],
                            lhsT=v_sb[:, jt, h, :],
                            rhs=e_bf[:, ic * 512:(ic + 1) * 512],
                            start=(jt == 0), stop=(jt == NT - 1))
                    if fillers:
                        fillers.pop(0)()

                # epilogue: attnU copy first (frees the PSUM accumulator
                # for the next head), then the denominator round-trip:
                # a [1,N] DVE reciprocal is single-lane (~6.5us!), so
                # bounce the row through DRAM reshaped to [128, N/128]
                # and run the reciprocal on all lanes (~0.2us).
                nc.vector.tensor_copy(
                    out=attnU[hb:hb + 64, t, :], in_=o_ps[0:HD, :])
                den = stage.tile([1, N], F32, tag="den")
                nc.vector.tensor_copy(out=den, in_=o_ps[HD:HD + 1, :])
                nc.sync.dma_start(out=dscratch[h:h + 1, :], in_=den)
                den128 = stage.tile([128, N // 128], F32, tag="den128")
                nc.sync.dma_start(out=den128, in_=dscratch[h, :].rearrange(
                    "(p a) -> p a", p=128))
                nc.vector.reciprocal(out=den128, in_=den128)
                nc.sync.dma_start(out=dscratch2[h, :].rearrange(
                    "(p a) -> p a", p=128), in_=den128)
                rbc = stage.tile([128, N], F32, tag="rbc")
                nc.sync.dma_start(out=rbc[hb:hb + 64, :],
                                  in_=bcast_ap(dscratch2[h, :], 64))
                if prev_mul is not None:
                    prev_mul()
                prev_mul = (lambda hb=hb, t=t, rbc=rbc: nc.vector.tensor_mul(
                    out=attnT[hb:hb + 64, t, :],
                    in0=attnU[hb:hb + 64, t, :], in1=rbc[hb:hb + 64, :]))
        prev_mul()

        if DEBUG_TAPS:
            taps = {
                "dbg_q": (qTt, BF16), "dbg_k": (kTt, BF16),
                "dbg_v": (v_sb, BF16), "dbg_attnU": (attnU, BF16),
                "dbg_attnT8": (attnT, BF16), "dbg_x8": (xT8, FP8),
                "dbg_w8": (wT8, FP8), "dbg_pw8": (pwT_bf, BF16),
            }
            for nm, (tl, dt) in taps.items():
                flat = int(np.prod(tl.shape[1:]))
                d = nc.dram_tensor(nm, [128, flat], dt,
                                   kind="ExternalOutput").ap()
                nc.sync.dma_start(out=d, in_=tl)
            dden = nc.dram_tensor("dbg_den", [H, N], F32,
                                  kind="ExternalOutput").ap()
            nc.sync.dma_start(out=dden, in_=dscratch)

        # ---- output projection (bf16) --------------------------------
        for t in range(NT):
            osb = outp.tile([128, C], F32, tag="osb")
            for lo, hi in ((0, 512), (512, 768)):
                psp = ps_q.tile([128, 512], F32, tag="ps1b", name="psp")
                for ct in range(CT):
                    nc.tensor.matmul(
                        psp[:, 0:hi - lo],
                        lhsT=attnT[:, ct, t * 128:(t + 1) * 128],
                        rhs=pwT_bf[:, lo // 128:hi // 128, ct, :],
                        start=(ct == 0), stop=(ct == CT - 1))
                nc.vector.tensor_add(
                    out=osb[:, lo:hi], in0=psp[:, 0:hi - lo],
                    in1=pjb_bc[:, lo:hi])
            eng = nc.sync if t % 2 == 0 else nc.scalar
            eng.dma_start(out=out.rearrange("(p t) c -> p t c", t=NT)[:, t, :],
                          in_=osb)

    _split_dma_waits(nc)
    return nc


_NC_CACHE = None


def _get_nc():
    global _NC_CACHE
    if _NC_CACHE is None:
        _NC_CACHE = build_kernel(
            bass.Bass("TRN2", target_bir_lowering=False, debug=False))
    return _NC_CACHE


def kernel(**inputs: np.ndarray) -> np.ndarray:
    nc = _get_nc()
    x = np.ascontiguousarray(inputs["x"], dtype=np.float32)
    shared = {
        "qkv_w": np.ascontiguousarray(inputs["qkv_w"], dtype=np.float32),
        "qkv_b": np.ascontiguousarray(inputs["qkv_b"], dtype=np.float32),
        "proj_w": np.ascontiguousarray(inputs["proj_w"], dtype=np.float32),
        "proj_b": np.ascontiguousarray(inputs["proj_b"], dtype=np.float32),
    }
    in_maps = [{"x": x[b], **shared} for b in range(B)]
    res = run_bass_kernel_spmd(nc, in_maps, core_ids=list(range(B)))
    return np.stack([r["out"] for r in res.results]).astype(np.float32)


if __name__ == "__main__":
    from reference import setup_inputs, reference

    inputs = {k: np.asarray(v) for k, v in setup_inputs().items()}
    got = kernel(**inputs)
    exp = np.asarray(reference(**inputs))
    err = np.abs(got - exp)
    print("abs err max:", err.max(), "ref absmax:", np.abs(exp).max())
    print("rel(absmax):", err.max() / np.abs(exp).max())
